# revision 1
# baseline (speedup 1.0000x reference)
"""Trainium2 Bass kernel for masked multi-adaptor LoRA:

    y = x @ W^T + b + sum_n mask[n] * SCALE * ((x @ A[n]^T) @ Bw[n]^T)

Strategy (8 NeuronCores, data-parallel over tokens):
  - Flatten x to [B*S, D] = [16384, 2048] tokens; each core takes T=2048 tokens.
  - Host pre-transposes/casts to bf16: xT [D, T] per core, WT = W^T [D, O],
    packed AT [128, KT*64], BwT [(n,r), O], m64[(n,r), t] = mask[n,t]*SCALE.
  - Device per core:
      hT[(n,r), t] = AT_k.T @ xT_k summed over k   (PE, runs along the xT DMA stream)
      gT = hT * m64  (DVE, cast bf16)  -> stored as rows 0:64 of a [128, T] tile
      y[t, o] = sum_{k=0..16} xk[k].T @ wk[k]      (PE)
    where k=16 is the LoRA tail: xk[16] = gT17 (rows 64:128 zeroed), wk[16] =
    BwT padded with zero rows — a uniform K=128 matmul, so the whole main loop
    is 17 homogeneous k-steps accumulating into PSUM.
  - First two token tiles run k-major (8 PSUM banks) so the PE follows the wT
    DMA stream; the rest run t-major from SBUF-resident data.
  - b is added on host (zeros in this problem, kept for generality).
"""

import os
import sys

if "/opt/trn_rl_repo" not in sys.path:
    sys.path.insert(0, "/opt/trn_rl_repo")

import numpy as np
import ml_dtypes

import concourse.mybir as mybir
import concourse.tile as tile
from concourse import bacc
from concourse.bass_utils import run_bass_kernel_spmd

N_CORES = 8
D = 2048          # d_in
O = 2048          # d_out
T = 2048          # tokens per core (16384 / 8)
NR = 64           # n_adaptors * r = 4 * 16
KT = D // 128     # 16 k-tiles
SCALE = 2.0       # lora_alpha / r = 32 / 16
XG = 8            # xT dma groups (2 k-tiles each)
WG = 8            # wT dma groups

FREE = 512        # moving-operand width (1024 fails: one matmul output <= one PSUM bank)
NOF = O // FREE   # output column tiles per token tile
NCH = T // FREE   # h-phase chunks
NTS = T // 128    # 128-token output row tiles

BF16 = mybir.dt.bfloat16
F32 = mybir.dt.float32

_NC = None


def _build():
    nc = bacc.Bacc("TRN2", target_bir_lowering=False, debug=False)
    xT = nc.dram_tensor("xT", [D, T], BF16, kind="ExternalInput").ap()
    wT = nc.dram_tensor("wT", [D, O], BF16, kind="ExternalInput").ap()
    aT = nc.dram_tensor("aT", [128, KT * NR], BF16, kind="ExternalInput").ap()
    bw17 = nc.dram_tensor("bw17", [NR, O], BF16, kind="ExternalInput").ap()
    m64 = nc.dram_tensor("m64", [NR, T], F32, kind="ExternalInput").ap()
    y = nc.dram_tensor("y", [T, O], F32, kind="ExternalOutput").ap()

    KX = KT // XG  # k-tiles per xT dma group
    KW = KT // WG

    with tile.TileContext(nc) as tc:
        with (
            tc.tile_pool(name="big", bufs=1) as big,
            tc.tile_pool(name="outp", bufs=3) as outp,
            tc.tile_pool(name="psum", bufs=8 * 512 // FREE, space="PSUM") as psum,
        ):
            # ---- resident loads; trigger order = arrival order ----
            aT_sb = big.tile([128, KT * NR], BF16, tag="aT_sb")
            nc.sync.dma_start(aT_sb, aT)

            m64_sb = big.tile([NR, T], F32, tag="m64_sb")
            nc.sync.dma_start(m64_sb, m64)

            wT17_sb = big.tile([128, O], BF16, tag="wT17_sb")
            nc.sync.dma_start(wT17_sb[0:NR, :], bw17)
            nc.gpsimd.memset(wT17_sb[NR:128, :], 0.0)

            gT17_sb = big.tile([128, T], BF16, tag="gT17_sb")
            nc.gpsimd.memset(gT17_sb[NR:128, :], 0.0)

            x_src = xT.rearrange("(g k p) t -> g p k t", g=XG, k=KX, p=128)
            xT_sb = []
            for g in range(XG):
                x_t = big.tile([128, KX * T], BF16, tag=f"xT{g}")
                nc.sync.dma_start(
                    x_t.rearrange("p (k t) -> p k t", k=KX), x_src[g]
                )
                xT_sb.append(x_t)

            w_src = wT.rearrange("(g k p) o -> g p k o", g=WG, k=KW, p=128)
            wT_sb = []
            for g in range(WG):
                w_t = big.tile([128, KW * O], BF16, tag=f"wT{g}")
                nc.sync.dma_start(
                    w_t.rearrange("p (k o) -> p k o", k=KW), w_src[g]
                )
                wT_sb.append(w_t)

            # slice helpers: k in [0, 16] with 16 = LoRA tail
            def xk(k, c0, c1):
                if k == KT:
                    return gT17_sb[:, c0:c1]
                return xT_sb[k // KX][:, (k % KX) * T + c0:(k % KX) * T + c1]

            def wk(k, c0, c1):
                if k == KT:
                    return wT17_sb[:, c0:c1]
                return wT_sb[k // KW][:, (k % KW) * O + c0:(k % KW) * O + c1]

            # ---- h phase (k-major, follows the xT stream) ----
            h_ps = [
                psum.tile([NR, FREE], F32, tag="ps", name=f"h_ps{c}")
                for c in range(NCH)
            ]
            for k in range(KT):
                a_sl = aT_sb[:, k * NR:(k + 1) * NR]
                for c in range(NCH):
                    nc.tensor.matmul(
                        h_ps[c],
                        a_sl,
                        xk(k, c * FREE, (c + 1) * FREE),
                        start=(k == 0),
                        stop=(k == KT - 1),
                    )

            # ---- g = h * (mask * SCALE) -> rows 0:64 of gT17 (bf16) ----
            for c in range(NCH):
                nc.vector.tensor_mul(
                    gT17_sb[0:NR, c * FREE:(c + 1) * FREE],
                    h_ps[c],
                    m64_sb[:, c * FREE:(c + 1) * FREE],
                )

            def drain(t, ys, split):
                ot = outp.tile([128, O], F32, tag="out", name=f"ot{t}")
                for o in range(NOF):
                    nc.vector.tensor_copy(ot[:, o * FREE:(o + 1) * FREE], ys[o])
                    if split:
                        nc.sync.dma_start(
                            y[t * 128:(t + 1) * 128, o * FREE:(o + 1) * FREE],
                            ot[:, o * FREE:(o + 1) * FREE],
                        )
                if not split:
                    nc.sync.dma_start(y[t * 128:(t + 1) * 128, :], ot)

            # ---- first two token tiles: k-major, follows the wT stream ----
            first = [
                [
                    psum.tile([128, FREE], F32, tag="ps", name=f"y_ps{t}_{o}")
                    for o in range(NOF)
                ]
                for t in range(2)
            ]
            for k in range(KT + 1):
                for t in range(2):
                    lhsT = xk(k, t * 128, (t + 1) * 128)
                    for o in range(NOF):
                        nc.tensor.matmul(
                            first[t][o],
                            lhsT,
                            wk(k, o * FREE, (o + 1) * FREE),
                            start=(k == 0),
                            stop=(k == KT),
                        )
            for t in range(2):
                drain(t, first[t], split=False)

            # ---- remaining token tiles: t-major from resident SBUF ----
            for t in range(2, NTS):
                ys = [
                    psum.tile([128, FREE], F32, tag="ps", name=f"y_ps{t}_{o}")
                    for o in range(NOF)
                ]
                for k in range(KT + 1):
                    lhsT = xk(k, t * 128, (t + 1) * 128)
                    for o in range(NOF):
                        nc.tensor.matmul(
                            ys[o],
                            lhsT,
                            wk(k, o * FREE, (o + 1) * FREE),
                            start=(k == 0),
                            stop=(k == KT),
                        )
                drain(t, ys, split=(t == NTS - 1))

    nc.compile()
    return nc


def _get_nc():
    global _NC
    if _NC is None:
        _NC = _build()
    return _NC


def _install_ntff_shim():
    """Optional: register the axon NTFF profile hook so trace=True works."""
    import types
    import antenv
    if "antenv.axon_hooks" in sys.modules:
        return
    hook = [None]
    mod = types.ModuleType("antenv.axon_hooks")
    mod.set_axon_ntff_profile_hook = lambda h: hook.__setitem__(0, h)
    mod.get_axon_ntff_profile_hook = lambda: hook[0]
    sys.modules["antenv.axon_hooks"] = mod
    antenv.axon_hooks = mod
    from trn_agent_boot.trn_boot import _ntff_profile_via_ctypes
    mod.set_axon_ntff_profile_hook(
        _ntff_profile_via_ctypes("/opt/axon/libaxon_pjrt.so")
    )
    from concourse import bass_utils
    bass_utils.upload_artifacts = lambda tmpdir: tmpdir


def kernel(x, mask, W, b, A, Bw):
    x = np.asarray(x)
    mask = np.asarray(mask)
    W = np.asarray(W)
    b = np.asarray(b)
    A = np.asarray(A)
    Bw = np.asarray(Bw)

    B_, S, _ = x.shape
    bf16 = ml_dtypes.bfloat16

    xt = x.reshape(B_ * S, D).astype(bf16)               # [16384, D]
    WT = np.ascontiguousarray(W.astype(bf16).T)          # [D, O]
    # packed A: aT[p, k*64+r] = A_cat[r, k*128+p]
    AT = np.ascontiguousarray(
        A.reshape(NR, KT, 128).transpose(2, 1, 0).reshape(128, KT * NR)
    ).astype(bf16)
    BWT = np.ascontiguousarray(
        Bw.transpose(0, 2, 1).reshape(NR, O).astype(bf16)
    )                                                    # [NR, O]
    m2 = (mask.reshape(mask.shape[0], -1) * np.float32(SCALE)).astype(np.float32)
    m64_full = np.repeat(m2, NR // mask.shape[0], axis=0)  # [NR, 16384]

    in_maps = []
    for c in range(N_CORES):
        sl = slice(c * T, (c + 1) * T)
        in_maps.append({
            "xT": np.ascontiguousarray(xt[sl].T),
            "wT": WT,
            "aT": AT,
            "bw17": BWT,
            "m64": np.ascontiguousarray(m64_full[:, sl]),
        })

    nc = _get_nc()
    trace = os.environ.get("KERNEL_TRACE") == "1"
    if trace:
        try:
            _install_ntff_shim()
        except Exception as e:  # profiling is best-effort
            print(f"NTFF shim unavailable: {e}", file=sys.stderr)
            trace = False
    res = run_bass_kernel_spmd(
        nc, in_maps, core_ids=list(range(N_CORES)), trace=trace
    )
    kernel.last_exec_time_ns = res.exec_time_ns
    kernel.last_trace = res.instructions_and_trace

    yf = np.concatenate([res.results[c]["y"] for c in range(N_CORES)], axis=0)
    yf = yf + b.astype(np.float32)[None, :]
    return yf.reshape(B_, S, O).astype(np.float32)



# revision 3
# speedup vs baseline: 1.0081x; 1.0081x over previous
"""Trainium2 Bass kernel for masked multi-adaptor LoRA:

    y = x @ W^T + b + sum_n mask[n] * SCALE * ((x @ A[n]^T) @ Bw[n]^T)

Strategy (8 NeuronCores, data-parallel over tokens), v2:
  - Flatten x to [B*S, D] = [16384, 2048]; each core takes T=2048 tokens.
  - Host pre-transposes/casts to bf16: xT [D, T], wT = W^T [D, O], packed
    aT [128, KT*64], w17 [128, O] = [BwT; BwT] (row-duplicated), m64
    [128, T] bf16 = mask*SCALE row-duplicated.  y is emitted bf16 and
    upcast on host (tolerance is 2e-2; bf16 rounding adds ~2e-3).
  - DMA order is consumption order on one HWDGE queue: aT, x k0-9,
    W k0-1, x k10-15, m64, w17, W k2-15.  The h-phase follows the x
    stream; a 2-tile k-major phase (tiles 0 and 4) follows the W stream.
  - h accumulators live in 2 PSUM banks: chunk c sits at row-half
    (c%2)*64 via column-placed matmuls, so the masked product g lands at
    alternating row-halves with no partition-crossing ops.
  - Remaining 14 token tiles run as 7 pairs chosen across chunk parity
    so the two LoRA tail matmuls (K=64) occupy disjoint PE row groups
    and execute concurrently (row tiling), halving the k=16 tail cost.
  - ~12 warmup matmuls on a zeroed tile run during the DMA ramp to lift
    the PE HAM clock gate to 8/8 before real work arrives.
  - Drain copies split across Vector and Scalar engines; the final tile
    drains per-512-column chunk to shorten the kernel tail.
"""

import os
import sys

if "/opt/trn_rl_repo" not in sys.path:
    sys.path.insert(0, "/opt/trn_rl_repo")

import numpy as np
import ml_dtypes

import concourse.mybir as mybir
import concourse.tile as tile
from concourse import bacc
from concourse.bass_utils import run_bass_kernel_spmd

N_CORES = 8
D = 2048          # d_in
O = 2048          # d_out
T = 2048          # tokens per core (16384 / 8)
NR = 64           # n_adaptors * r = 4 * 16
KT = D // 128     # 16 k-tiles
SCALE = 2.0       # lora_alpha / r = 32 / 16
XG = 8            # xT dma groups (2 k-tiles each)
WG = 8            # wT dma groups
KX = KT // XG
KW = KT // WG

FREE = 512        # moving-operand width (one matmul output <= one PSUM bank)
NOF = O // FREE   # output column tiles per token tile
NCH = T // FREE   # h-phase chunks
NTS = T // 128    # 128-token output row tiles
WARMUP_MMS = 12

BF16 = mybir.dt.bfloat16
F32 = mybir.dt.float32

# token-tile pairs: first2 is k-major along the W stream; the rest are
# t-major.  Pair members come from opposite chunk parity so their g rows
# sit at different row-halves (0:64 vs 64:128) for concurrent tails.
FIRST2 = (0, 4)
PAIRS = [(1, 5), (2, 6), (3, 7), (8, 12), (9, 13), (10, 14), (11, 15)]

_NC = None


def _build():
    nc = bacc.Bacc("TRN2", target_bir_lowering=False, debug=False)
    xT = nc.dram_tensor("xT", [D, T], BF16, kind="ExternalInput").ap()
    wT = nc.dram_tensor("wT", [D, O], BF16, kind="ExternalInput").ap()
    aT = nc.dram_tensor("aT", [128, KT * NR], BF16, kind="ExternalInput").ap()
    w17 = nc.dram_tensor("w17", [128, O], BF16, kind="ExternalInput").ap()
    m64 = nc.dram_tensor("m64", [128, T], BF16, kind="ExternalInput").ap()
    y = nc.dram_tensor("y", [T, O], BF16, kind="ExternalOutput").ap()

    with tile.TileContext(nc) as tc:
        with (
            tc.tile_pool(name="big", bufs=1) as big,
            tc.tile_pool(name="outp", bufs=3) as outp,
            tc.tile_pool(name="psum", bufs=8, space="PSUM") as psum,
        ):
            # ---- warmup source (zeroed) + h accumulators ----
            warm = big.tile([128, 64 + FREE], BF16, tag="warm")
            nc.vector.memset(warm, 0.0)

            hA = psum.tile([128, FREE], F32, tag="ps", name="hA")
            hB = psum.tile([128, FREE], F32, tag="ps", name="hB")

            for _ in range(WARMUP_MMS):
                nc.tensor.matmul(
                    hA[0:64, :],
                    warm[:, 0:64],
                    warm[:, 64:64 + FREE],
                    start=True,
                    stop=True,
                    skip_group_check=True,
                )

            # ---- resident loads; trigger order = arrival order ----
            aT_sb = big.tile([128, KT * NR], BF16, tag="aT_sb")
            nc.sync.dma_start(aT_sb, aT)

            x_src = xT.rearrange("(g k p) t -> g p k t", g=XG, k=KX, p=128)
            xT_sb = [
                big.tile([128, KX * T], BF16, tag=f"xT{g}", name=f"xT{g}")
                for g in range(XG)
            ]
            w_src = wT.rearrange("(g k p) o -> g p k o", g=WG, k=KW, p=128)
            wT_sb = [
                big.tile([128, KW * O], BF16, tag=f"wT{g}", name=f"wT{g}")
                for g in range(WG)
            ]

            def x_dma(g):
                nc.sync.dma_start(
                    xT_sb[g].rearrange("p (k t) -> p k t", k=KX), x_src[g]
                )

            def w_dma(g):
                nc.sync.dma_start(
                    wT_sb[g].rearrange("p (k o) -> p k o", k=KW), w_src[g]
                )

            m64_sb = big.tile([128, T], BF16, tag="m64_sb")
            w17_sb = big.tile([128, O], BF16, tag="w17_sb")
            gT17_sb = big.tile([128, T], BF16, tag="gT17_sb")

            for g in range(5):
                x_dma(g)          # x k0..k9
            w_dma(0)              # W k0,k1
            for g in range(5, XG):
                x_dma(g)          # x k10..k15
            nc.sync.dma_start(m64_sb, m64)
            nc.sync.dma_start(w17_sb, w17)
            for g in range(1, WG):
                w_dma(g)          # W k2..k15

            def xk(k, c0, c1):
                g, j = k // KX, k % KX
                return xT_sb[g][:, j * T + c0:j * T + c1]

            def wk(k, c0, c1):
                g, j = k // KW, k % KW
                return wT_sb[g][:, j * O + c0:j * O + c1]

            # h chunk c lives at row-half (c%2)*64 of hA (c<2) / hB
            def h_out(c):
                t_ = hA if c < 2 else hB
                r0 = (c % 2) * 64
                return t_[r0:r0 + 64, :]

            # g slice used by the k=16 tail of token tile t
            def g_sl(t):
                r0 = ((t // 4) % 2) * 64
                return gT17_sb[r0:r0 + 64, t * 128:(t + 1) * 128]

            def w17_sl(t, o):
                r0 = ((t // 4) % 2) * 64
                return w17_sb[r0:r0 + 64, o * FREE:(o + 1) * FREE]

            # ---- h phase (k-major, follows the xT stream) ----
            for k in range(KT):
                a_sl = aT_sb[:, k * NR:(k + 1) * NR]
                for c in range(NCH):
                    nc.tensor.matmul(
                        h_out(c),
                        a_sl,
                        xk(k, c * FREE, (c + 1) * FREE),
                        start=(k == 0),
                        stop=(k == KT - 1),
                    )

            # ---- g = h * (mask * SCALE), bf16, at chunk-parity row-half ----
            for c in range(NCH):
                r0 = (c % 2) * 64
                nc.vector.tensor_mul(
                    gT17_sb[r0:r0 + 64, c * FREE:(c + 1) * FREE],
                    h_out(c),
                    m64_sb[r0:r0 + 64, c * FREE:(c + 1) * FREE],
                )

            def drain(t, ys, split):
                ot = outp.tile([128, O], BF16, tag="out", name=f"ot{t}")
                for o in range(NOF):
                    dst = ot[:, o * FREE:(o + 1) * FREE]
                    if o < 2:
                        nc.vector.tensor_copy(dst, ys[o])
                    else:
                        nc.scalar.copy(dst, ys[o])
                    if split:
                        nc.sync.dma_start(
                            y[t * 128:(t + 1) * 128, o * FREE:(o + 1) * FREE],
                            dst,
                        )
                if not split:
                    nc.sync.dma_start(y[t * 128:(t + 1) * 128, :], ot)

            def tails(ta, ys_a, tb, ys_b, start_b):
                # ta's tail at rows r0(ta), tb's at the other half: the
                # two matmuls target disjoint PE row groups and overlap.
                for o in range(NOF):
                    nc.tensor.matmul(
                        ys_a[o], g_sl(ta), w17_sl(ta, o), start=False, stop=True
                    )
                    nc.tensor.matmul(
                        ys_b[o], g_sl(tb), w17_sl(tb, o), start=start_b, stop=not start_b
                    )

            # ---- first two token tiles: k-major, follows the wT stream ----
            ta, tb = FIRST2
            ysA = [psum.tile([128, FREE], F32, tag="ps", name=f"y{ta}_{o}")
                   for o in range(NOF)]
            ysB = [psum.tile([128, FREE], F32, tag="ps", name=f"y{tb}_{o}")
                   for o in range(NOF)]
            for k in range(KT):
                for t, ys in ((ta, ysA), (tb, ysB)):
                    lhsT = xk(k, t * 128, (t + 1) * 128)
                    for o in range(NOF):
                        nc.tensor.matmul(
                            ys[o],
                            lhsT,
                            wk(k, o * FREE, (o + 1) * FREE),
                            start=(k == 0),
                            stop=False,
                        )
            tails(ta, ysA, tb, ysB, start_b=False)
            drain(ta, ysA, split=False)
            drain(tb, ysB, split=False)

            # ---- remaining tiles: t-major pairs with concurrent tails ----
            for pi, (ta, tb) in enumerate(PAIRS):
                ysA = [psum.tile([128, FREE], F32, tag="ps", name=f"y{ta}_{o}")
                       for o in range(NOF)]
                ysB = [psum.tile([128, FREE], F32, tag="ps", name=f"y{tb}_{o}")
                       for o in range(NOF)]
                for k in range(KT):
                    lhsT = xk(k, ta * 128, (ta + 1) * 128)
                    for o in range(NOF):
                        nc.tensor.matmul(
                            ysA[o],
                            lhsT,
                            wk(k, o * FREE, (o + 1) * FREE),
                            start=(k == 0),
                            stop=False,
                        )
                # ta's stop-tail paired with tb's start-tail (row groups
                # 0:64 and 64:128 run concurrently)
                tails(ta, ysA, tb, ysB, start_b=True)
                drain(ta, ysA, split=False)
                for k in range(KT):
                    lhsT = xk(k, tb * 128, (tb + 1) * 128)
                    for o in range(NOF):
                        nc.tensor.matmul(
                            ysB[o],
                            lhsT,
                            wk(k, o * FREE, (o + 1) * FREE),
                            start=False,
                            stop=(k == KT - 1),
                        )
                drain(tb, ysB, split=(pi == len(PAIRS) - 1))

    nc.compile()
    return nc


def _get_nc():
    global _NC
    if _NC is None:
        _NC = _build()
    return _NC


def _install_ntff_shim():
    """Optional: register the axon NTFF profile hook so trace=True works."""
    import types
    import antenv
    if "antenv.axon_hooks" in sys.modules:
        return
    hook = [None]
    mod = types.ModuleType("antenv.axon_hooks")
    mod.set_axon_ntff_profile_hook = lambda h: hook.__setitem__(0, h)
    mod.get_axon_ntff_profile_hook = lambda: hook[0]
    sys.modules["antenv.axon_hooks"] = mod
    antenv.axon_hooks = mod
    from trn_agent_boot.trn_boot import _ntff_profile_via_ctypes
    mod.set_axon_ntff_profile_hook(
        _ntff_profile_via_ctypes("/opt/axon/libaxon_pjrt.so")
    )
    from concourse import bass_utils
    bass_utils.upload_artifacts = lambda tmpdir: tmpdir


def kernel(x, mask, W, b, A, Bw):
    x = np.asarray(x)
    mask = np.asarray(mask)
    W = np.asarray(W)
    b = np.asarray(b)
    A = np.asarray(A)
    Bw = np.asarray(Bw)

    B_, S, _ = x.shape
    bf16 = ml_dtypes.bfloat16

    xt = x.reshape(B_ * S, D).astype(bf16)               # [16384, D]
    WT = np.ascontiguousarray(W.astype(bf16).T)          # [D, O]
    # packed A: aT[p, k*64+r] = A_cat[r, k*128+p]
    AT = np.ascontiguousarray(
        A.reshape(NR, KT, 128).transpose(2, 1, 0).reshape(128, KT * NR)
    ).astype(bf16)
    BWT = Bw.transpose(0, 2, 1).reshape(NR, O)           # [NR, O]
    W17 = np.ascontiguousarray(
        np.concatenate([BWT, BWT], axis=0).astype(bf16)
    )                                                    # [128, O]
    m2 = mask.reshape(mask.shape[0], -1) * np.float32(SCALE)
    m64_full = np.repeat(m2, NR // mask.shape[0], axis=0)   # [NR, 16384]
    m128_full = np.ascontiguousarray(
        np.concatenate([m64_full, m64_full], axis=0).astype(bf16)
    )                                                    # [128, 16384]

    in_maps = []
    for c in range(N_CORES):
        sl = slice(c * T, (c + 1) * T)
        in_maps.append({
            "xT": np.ascontiguousarray(xt[sl].T),
            "wT": WT,
            "aT": AT,
            "w17": W17,
            "m64": np.ascontiguousarray(m128_full[:, sl]),
        })

    nc = _get_nc()
    trace = os.environ.get("KERNEL_TRACE") == "1"
    if trace:
        try:
            _install_ntff_shim()
        except Exception as e:  # profiling is best-effort
            print(f"NTFF shim unavailable: {e}", file=sys.stderr)
            trace = False
    res = run_bass_kernel_spmd(
        nc, in_maps, core_ids=list(range(N_CORES)), trace=trace
    )
    kernel.last_exec_time_ns = res.exec_time_ns
    kernel.last_trace = res.instructions_and_trace

    yf = np.concatenate(
        [res.results[c]["y"].astype(np.float32) for c in range(N_CORES)], axis=0
    )
    yf = yf + b.astype(np.float32)[None, :]
    return yf.reshape(B_, S, O).astype(np.float32)
